# revision 8
# baseline (speedup 1.0000x reference)
"""Causal self-attention on 8 TRN2 NeuronCores.

Problem: x[2,2048,1024], wq/wk/wv/wo[1024,1024] (nn.Linear convention,
out = y @ W.T), H=16 heads, D=64, causal softmax, f32.

Sharding: tensor-parallel over heads x data-parallel over batch.
Core i handles batch b=i//4 and head group g=i%4 (4 heads each).
wq/wk/wv are split row-wise (output-feature) per head group; wo is
split column-wise; each core returns a partial output projection
out_partial[b] and the host sums the 4 partials per batch.

On-device layout is feature-major (transposed): the host passes
xT=x[b].T etc so every matmul sees its contraction dim on SBUF
partitions and no on-device transposes are needed.

This version is scheduled for PE clock density: the TRN2 tensor engine
runs at 2.4 GHz only while its activity stays dense (HAM p-state);
gaps drop it to 1.2 GHz.  Structure:
  - dummy warmup matmuls ramp the PE while x streams in;
  - attention is processed in half-head units (2 query spans, so PV
    needs only 2 PSUM banks), leaving 2 PSUM banks for *filler*
    matmuls (V projections, KQ m=1 projections, and the output
    projection split into its two 128-row contraction halves) that
    are injected between attention key-chunks so the PE never waits
    for the softmax exp on ScalarE;
  - softmax normalization: DVE reciprocal on the ones-column row-sum
    [1,512], then a K=1 ones-stationary matmul broadcasts it across
    64 partitions in PSUM (no DRAM roundtrips, no gpsimd SWDGE);
  - causal masking via gpsimd affine_select on the exp output;
  - PSUM->SBUF copies for K/Q projections run on the (then idle)
    ScalarE; everything else elementwise on DVE.
"""

import sys

for _p in ("/opt/trn_rl_repo", "/root/.axon_site"):
    if _p not in sys.path:
        sys.path.insert(0, _p)

import numpy as np

import concourse.bass as bass
import concourse.mybir as mybir
import concourse.tile as tile
from concourse import bacc
from concourse.bass_utils import run_bass_kernel_spmd

B, T, C, H = 2, 2048, 1024, 16
DH = C // H            # 64 head dim
HG = 4                 # heads per core
GW = HG * DH           # 256 features per head group
NB = T // 128          # 16 key chunks
NS = T // 512          # 4 query spans
KC = C // 128          # 8 contraction chunks over C
SCALE = 1.0 / float(np.sqrt(DH))
N_CORES = 8
N_WARM = 16            # dummy PE warmup matmuls

F32 = mybir.dt.float32
F32R = mybir.dt.float32r
BF16 = mybir.dt.bfloat16
EXP = mybir.ActivationFunctionType.Exp
COPY = mybir.ActivationFunctionType.Copy


def build_nc():
    nc = bacc.Bacc("TRN2", target_bir_lowering=False, debug=False,
                   num_devices=N_CORES)
    xT = nc.declare_dram_parameter("xT", [C, T], F32R, isOutput=False)
    wqT = nc.declare_dram_parameter("wqT", [C, GW], F32R, isOutput=False)
    wkT = nc.declare_dram_parameter("wkT", [C, GW], F32R, isOutput=False)
    wvT = nc.declare_dram_parameter("wvT", [C, GW], F32R, isOutput=False)
    woT = nc.declare_dram_parameter("woT", [GW, C], F32R, isOutput=False)
    outT = nc.declare_dram_parameter("outT", [C, T], F32, isOutput=True)
    wscr = nc.dram_tensor("w_scratch", [128, 4], F32)

    with tile.TileContext(nc) as tc:
        with tc.tile_pool(name="pers", bufs=1) as pers, \
             tc.tile_pool(name="mgs", bufs=2, space="PSUM") as mgs, \
             tc.tile_pool(name="pvs", bufs=1, space="PSUM") as pvs, \
             tc.tile_pool(name="aux", bufs=2, space="PSUM") as aux, \
             tc.tile_pool(name="pts", bufs=3) as pts, \
             tc.tile_pool(name="nrm", bufs=3) as nrm, \
             tc.tile_pool(name="ost", bufs=4) as ost:

            # ---------- PE warmup: dense dummy matmuls from t~0 ----------
            dumx = pers.tile([128, 512], BF16, tag="dumx", name="dumx")
            nc.gpsimd.memset(dumx, 0.0)
            wps = aux.tile([128, 512], F32, tag="aux", name="wps")
            for i in range(N_WARM):
                nc.tensor.matmul(wps, dumx[:, 0:128], dumx,
                                 start=(i == 0), stop=(i == N_WARM - 1))
            dsb = pers.tile([128, 4], F32, tag="dsb", name="dsb")
            nc.vector.tensor_copy(out=dsb, in_=wps[:, 0:4])
            nc.sync.dma_start(out=wscr[:, :], in_=dsb)

            # ---------- persistent SBUF; DMAs in consumption order -------
            wk_t = [pers.tile([128, GW], F32R, tag=f"wk{i}", name=f"wk{i}")
                    for i in range(KC)]
            for i in range(KC):
                nc.sync.dma_start(out=wk_t[i], in_=wkT[i * 128:(i + 1) * 128, :])
            wq_t = [pers.tile([128, GW], F32R, tag=f"wq{i}", name=f"wq{i}")
                    for i in range(KC)]
            wv_t = [pers.tile([128, GW], F32R, tag=f"wv{i}", name=f"wv{i}")
                    for i in range(KC)]
            wo_t = [pers.tile([128, C], F32R, tag=f"wo{j}", name=f"wo{j}")
                    for j in range(2)]

            qts = [pers.tile([128, T], F32R, tag=f"qT{m}", name=f"qT{m}")
                   for m in range(2)]
            kts = [pers.tile([128, T], F32R, tag=f"kT{m}", name=f"kT{m}")
                   for m in range(2)]
            yts = [pers.tile([128, T], F32R, tag=f"yT{m}", name=f"yT{m}")
                   for m in range(2)]
            vts = [pers.tile([128, HG * 65], BF16, tag=f"V{tb}", name=f"V{tb}")
                   for tb in range(NB)]

            # ones for the V ones-columns and the 1->64 sum broadcast
            ones4 = pers.tile([128, 4], BF16, tag="ones4", name="ones4")
            for j in range(4):
                nc.scalar.activation(
                    out=ones4[:, j:j + 1],
                    in_=nc.const_aps.tensor(1.0, [128, 1]), func=COPY)
            ones1 = pers.tile([1, 64], F32R, tag="ones1", name="ones1")
            nc.scalar.activation(
                out=ones1, in_=nc.const_aps.tensor(1.0, [1, 64]), func=COPY)

            # ---------------- emission helpers ----------------
            def kq_group(wt, dst, m, s):
                """One K/Q projection span-chunk: 8 matmuls + copy."""
                ps = aux.tile([128, 512], F32, tag="aux", name="kqps")
                for k in range(KC):
                    nc.tensor.matmul(
                        ps,
                        wt[k][:, m * 128:(m + 1) * 128],
                        xts[k][:, s * 512:(s + 1) * 512],
                        start=(k == 0), stop=(k == KC - 1))
                if m == 0:   # phase 1: ScalarE is idle
                    nc.scalar.activation(
                        out=dst[m][:, s * 512:(s + 1) * 512], in_=ps, func=COPY)
                else:        # phase 2 filler: ScalarE busy with exp
                    nc.vector.tensor_copy(
                        out=dst[m][:, s * 512:(s + 1) * 512], in_=ps)

            def v_group(tb):
                vps = aux.tile([128, GW], F32, tag="aux", name="vps")
                for k in range(KC):
                    nc.tensor.matmul(
                        vps, xts[k][:, tb * 128:(tb + 1) * 128], wv_t[k],
                        start=(k == 0), stop=(k == KC - 1))
                vt = vts[tb]
                nc.vector.tensor_copy(
                    out=vt.rearrange("p (h c) -> p h c", c=65)[:, :, 0:64],
                    in_=vps.rearrange("p (h c) -> p h c", c=64))
                nc.vector.tensor_copy(
                    out=vt.rearrange("p (h c) -> p h c", c=65)[:, :, 64],
                    in_=ones4)

            z0 = {}   # (m, s) -> staged wo-j0 partial in SBUF
            zref = {}  # z pool, opened after the x pool closes

            def j0_pair(ms_list):
                for m, s in ms_list:
                    op = aux.tile([128, 512], F32, tag="aux", name="j0op")
                    nc.tensor.matmul(
                        op, wo_t[0][:, m * 128:(m + 1) * 128],
                        yts[0][:, s * 512:(s + 1) * 512],
                        start=True, stop=True)
                    zt = zref["p"].tile([128, 512], F32, tag=f"z{m}_{s}",
                                        name=f"z{m}_{s}")
                    z0[(m, s)] = zt
                    nc.vector.tensor_copy(out=zt, in_=op)

            def j1_pair(ms_list):
                for m, s in ms_list:
                    op = aux.tile([128, 512], F32, tag="aux", name="j1op")
                    nc.tensor.matmul(
                        op, wo_t[1][:, m * 128:(m + 1) * 128],
                        yts[1][:, s * 512:(s + 1) * 512],
                        start=True, stop=True)
                    ot = ost.tile([128, 512], F32, tag="ot", name="ot")
                    nc.vector.tensor_add(out=ot, in0=z0[(m, s)], in1=op)
                    nc.sync.dma_start(
                        out=outT[m * 128:(m + 1) * 128, s * 512:(s + 1) * 512],
                        in_=ot)

            pending = []   # deferred normalize closures (popped 1 ki later)

            def finalize(h, sg, pv, yt, po):
                """Span done: copy out of PSUM, recip of the sums row, then
                (deferred) ones-broadcast matmul + normalize mul."""
                yv = nrm.tile([65, 512], F32, tag="yv", name="yv")
                nc.vector.tensor_copy(out=yv, in_=pv[0:65, :])
                rr = nrm.tile([1, 512], F32R, tag="rr", name="rr")
                with nc.allow_low_precision(
                        reason="f32r is fp32-width; needed as matmul rhs"):
                    nc.vector.reciprocal(out=rr, in_=yv[64:65, :])

                def _bcast_mul():
                    rps = aux.tile([64, 512], F32, tag="aux", name="rps")
                    nc.tensor.matmul(rps, ones1, rr, start=True, stop=True)
                    nc.vector.tensor_mul(
                        out=yt[po:po + 64, sg * 512:(sg + 1) * 512],
                        in0=yv[0:64, :], in1=rps)
                pending.append(_bcast_mul)

            def unit(h, uh, fillers):
                """Attention for head h, query half uh (2 spans)."""
                qt, kt, yt = qts[h // 2], kts[h // 2], yts[h // 2]
                po = (h % 2) * 64
                q0 = uh * 1024
                spans = (2 * uh, 2 * uh + 1)
                n_ki = 8 if uh == 0 else 16
                pv_t = [pvs.tile([65, 512], F32, tag=f"pv{sp}", name=f"pv{sp}")
                        for sp in range(2)]
                for ki in range(n_ki):
                    dcol = ki * 128 - q0
                    lo = max(dcol, 0)
                    mg = mgs.tile([128, 1024], F32, tag="mg", name="mg")
                    for sp in range(2):
                        a = max(lo, sp * 512)
                        b = (sp + 1) * 512
                        if a < b:
                            nc.tensor.matmul(
                                mg[:, a:b],
                                kt[po:po + 64, ki * 128:(ki + 1) * 128],
                                qt[po:po + 64, q0 + a:q0 + b],
                                start=True, stop=True)
                    pt = pts.tile([128, 1024], BF16, tag="pt", name="pt")
                    nc.scalar.activation(
                        out=pt[:, lo:1024], in_=mg[:, lo:1024],
                        func=EXP, scale=SCALE)
                    if dcol >= 0:
                        nc.gpsimd.affine_select(
                            out=pt[:, dcol:dcol + 128],
                            in_=pt[:, dcol:dcol + 128],
                            compare_op=mybir.AluOpType.is_ge,
                            fill=0.0, base=0, pattern=[[1, 128]],
                            channel_multiplier=-1)
                    # deferred normalizes + scheduled filler for this ki
                    while pending:
                        pending.pop(0)()
                    for f in fillers.get(ki, ()):
                        f()
                    for sp in range(2):
                        sg = spans[sp]
                        last = 4 * sg + 3
                        a = max(lo, sp * 512)
                        b = (sp + 1) * 512
                        if ki <= last and a < b:
                            nc.tensor.matmul(
                                pv_t[sp][:, a - sp * 512:512],
                                vts[ki][:, h * 65:(h + 1) * 65],
                                pt[:, a:b],
                                start=(ki == 0), stop=(ki == last))
                        if ki == last:
                            finalize(h, sg, pv_t[sp], yt, po)

            # ================= phase A (xts live) =================
            with tc.tile_pool(name="xtp", bufs=1) as xtp:
                xts = [xtp.tile([128, T], F32R, tag=f"xT{i}", name=f"xT{i}")
                       for i in range(KC)]
                for i in range(KC):
                    nc.sync.dma_start(
                        out=xts[i][:, 0:512], in_=xT[i * 128:(i + 1) * 128, 0:512])
                for i in range(KC):
                    nc.sync.dma_start(out=wq_t[i],
                                      in_=wqT[i * 128:(i + 1) * 128, :])
                for s in range(1, NS):
                    for i in range(KC):
                        nc.sync.dma_start(
                            out=xts[i][:, s * 512:(s + 1) * 512],
                            in_=xT[i * 128:(i + 1) * 128, s * 512:(s + 1) * 512])
                for i in range(KC):
                    nc.sync.dma_start(out=wv_t[i],
                                      in_=wvT[i * 128:(i + 1) * 128, :])
                for j in range(2):
                    nc.sync.dma_start(out=wo_t[j],
                                      in_=woT[j * 128:(j + 1) * 128, :])

                # K/Q m=0 projections (heads 0,1), span-major
                for s in range(NS):
                    kq_group(wk_t, kts, 0, s)
                    kq_group(wq_t, qts, 0, s)
                # V projections for key chunks 0..7
                for tb in range(8):
                    v_group(tb)
                # head 0 half 1; V 8..15 injected as filler
                unit(0, 0, {i: (lambda tb=tb: v_group(tb),)
                            for i, tb in enumerate(range(8, 16))})

            # ================= phase B =================
            zpl = tc.tile_pool(name="zpl", bufs=1)
            zref["p"] = zpl.__enter__()
            unit(0, 1, {3: (lambda: kq_group(wk_t, kts, 1, 0),),
                        7: (lambda: kq_group(wq_t, qts, 1, 0),),
                        11: (lambda: kq_group(wk_t, kts, 1, 1),),
                        15: (lambda: kq_group(wq_t, qts, 1, 1),)})
            unit(1, 0, {2: (lambda: kq_group(wk_t, kts, 1, 2),),
                        5: (lambda: kq_group(wq_t, qts, 1, 2),)})
            unit(1, 1, {5: (lambda: kq_group(wk_t, kts, 1, 3),),
                        11: (lambda: kq_group(wq_t, qts, 1, 3),)})
            unit(2, 0, {1: (lambda: j0_pair([(0, 0), (1, 0)]),),
                        3: (lambda: j0_pair([(2, 0), (3, 0)]),),
                        5: (lambda: j0_pair([(4, 0), (5, 0)]),),
                        7: (lambda: j0_pair([(6, 0), (7, 0)]),)})
            unit(2, 1, {1: (lambda: j0_pair([(0, 1), (1, 1)]),),
                        3: (lambda: j0_pair([(2, 1), (3, 1)]),),
                        5: (lambda: j0_pair([(4, 1), (5, 1)]),),
                        7: (lambda: j0_pair([(6, 1), (7, 1)]),),
                        9: (lambda: j0_pair([(0, 2), (1, 2)]),),
                        11: (lambda: j0_pair([(2, 2), (3, 2)]),),
                        13: (lambda: j0_pair([(4, 2), (5, 2)]),),
                        15: (lambda: j0_pair([(6, 2), (7, 2)]),)})
            unit(3, 0, {0: (lambda: j0_pair([(0, 3), (1, 3)]),),
                        1: (lambda: j0_pair([(2, 3), (3, 3)]),),
                        2: (lambda: j0_pair([(4, 3), (5, 3)]),),
                        3: (lambda: j0_pair([(6, 3), (7, 3)]),),
                        5: (lambda: j1_pair([(0, 0), (1, 0), (2, 0)]),),
                        6: (lambda: j1_pair([(3, 0), (4, 0), (5, 0)]),),
                        7: (lambda: j1_pair([(6, 0), (7, 0)]),)})
            unit(3, 1, {1: (lambda: j1_pair([(0, 1), (1, 1)]),),
                        3: (lambda: j1_pair([(2, 1), (3, 1)]),),
                        5: (lambda: j1_pair([(4, 1), (5, 1)]),),
                        7: (lambda: j1_pair([(6, 1), (7, 1)]),),
                        13: (lambda: j1_pair([(0, 2), (1, 2)]),),
                        14: (lambda: j1_pair([(2, 2), (3, 2)]),),
                        15: (lambda: j1_pair([(4, 2), (5, 2)]),)})
            while pending:
                pending.pop(0)()
            j1_pair([(6, 2), (7, 2)])
            j1_pair([(m, 3) for m in range(8)])
            zpl.__exit__(None, None, None)
    nc.compile()
    return nc


_NC_CACHE = None


def _get_nc():
    global _NC_CACHE
    if _NC_CACHE is None:
        _NC_CACHE = build_nc()
    return _NC_CACHE


def make_in_maps(x, wq, wk, wv, wo):
    x = np.asarray(x, dtype=np.float32)
    wq = np.asarray(wq, dtype=np.float32)
    wk = np.asarray(wk, dtype=np.float32)
    wv = np.asarray(wv, dtype=np.float32)
    wo = np.asarray(wo, dtype=np.float32)
    in_maps = []
    for core in range(N_CORES):
        b, g = core // HG, core % HG
        rows = slice(g * GW, (g + 1) * GW)
        in_maps.append({
            "xT": np.ascontiguousarray(x[b].T),
            "wqT": np.ascontiguousarray(wq[rows, :].T),
            "wkT": np.ascontiguousarray(wk[rows, :].T),
            "wvT": np.ascontiguousarray(wv[rows, :].T),
            "woT": np.ascontiguousarray(wo[:, rows].T),
        })
    return in_maps


def run(x, wq, wk, wv, wo, trace=False, tmpdir=None):
    nc = _get_nc()
    in_maps = make_in_maps(x, wq, wk, wv, wo)
    res = run_bass_kernel_spmd(nc, in_maps, core_ids=list(range(N_CORES)),
                               trace=trace, tmpdir=tmpdir)
    out = np.zeros((B, T, C), dtype=np.float32)
    for core in range(N_CORES):
        out[core // HG] += res.results[core]["outT"].T
    return out, res


def kernel(x, wq, wk, wv, wo):
    out, _ = run(x, wq, wk, wv, wo)
    return out
